# revision 18
# baseline (speedup 1.0000x reference)
"""Trainium2 Bass kernel for nn_DecoderAttention (bilinear-score attention).

Computes, for full inputs h_d_t [32,1024], h_d_all [32,4096,1024], W [1024,1024]:
    qW    = h_d_t @ W
    e     = einsum('bd,btd->bt', qW, h_d_all)
    alpha = exp(e) / (sum(e, axis=1) + 1e-8)
    c_t   = einsum('bt,btd->bd', alpha, h_d_all)

Strategy: data-parallel over batch — 4 batches per NeuronCore across 8 cores,
W replicated. Single pass over the cache, streamed as 4 MiB slabs (8 t-tiles
per DMA, tapering at the end of the stream) on the SP HWDGE ring, which
carries ONLY the slab stream: HWDGE rings execute in order, so any DMA that
waits on compute (W/hdt staging, qW rows, outputs) would head-of-line block
the stream — those all go through the GPSIMD SWDGE queue instead. Measured
214us vs 221us (slabs alternating SP+ACT) vs 233us (everything on SP); the
original 512 KiB-per-DMA version ran at ~139 GB/s/core vs ~360 here. h lives in DRAM and SBUF as float32r (bit-identical to f32) so
the TensorEngine fast path needs no per-tile conversion copy; DVE reads the
same bytes for the scores. Per 128-row t-tile, a fused DVE multiply+reduce
produces the raw scores, ACT exponentiates a slab's worth at a time, and the
TensorEngine accumulates exp(e)^T @ h into PSUM. The denominator is the
raw-score running sum, applied once per batch, so h_d_all is read exactly
once (64 MiB + 4 MiB of W per core ≈ 181 us of HBM at the measured
376 GB/s/core — the memory-roofline floor for this kernel).
"""

import numpy as np

import concourse.bass as bass  # noqa: F401  (engine types pulled via bacc)
import concourse.mybir as mybir
import concourse.tile as tile
from concourse import bacc, bass_utils

B, T, D = 32, 4096, 1024
N_CORES = 8
B_LOC = B // N_CORES  # 4 batches per core
TT = 128              # t-tile rows (matmul contraction dim)
NT = T // TT          # 32 t-tiles per batch
SLAB = 8              # t-tiles per DMA slab (4 MiB)
NSLAB = NT // SLAB    # 4 slabs per batch
EXP_GRAN = 2          # tiles per exp/matmul group within a slab
# slab DMA queue: SP ring carries ONLY the h slab stream (in-order HWDGE
# rings head-of-line block on any DMA that waits for compute, so W/hdt/qrow/c
# all go via SWDGE instead)
DMA_ENGS = [lambda nc: nc.sync]
SLABS_STD = [8, 8, 8, 8]       # t-tiles per slab DMA, ordinary batches
SLABS_LAST = [8, 8, 8, 4, 2, 1, 1]  # taper the final batch so the post-stream
                                 # tail is ~2 tiles of compute, not 8
EPS = 1e-8

_NC_CACHE = {}


def _build_module(reps=1):
    f32 = mybir.dt.float32
    f32r = mybir.dt.float32r

    nc = bacc.Bacc("TRN2", debug=False, num_devices=N_CORES)
    h_d = nc.dram_tensor("h", [B_LOC, T, D], f32r, kind="ExternalInput")
    hdtT_d = nc.dram_tensor("hdtT", [D, B_LOC], f32, kind="ExternalInput")
    w_d = nc.dram_tensor("W", [D, D], f32, kind="ExternalInput")
    c_d = nc.dram_tensor("c", [B_LOC, D], f32, kind="ExternalOutput")

    # tile-major view: [b, p, i, d] with t = i*TT + p; a slab DMA takes an
    # i-range so slab sizes can taper at the end of the stream
    h_ap = h_d.ap().rearrange("b (i p) d -> b p i d", p=TT)      # [4, 128, 32, 1024]
    w_ap = w_d.ap().rearrange("(c p) j -> p c j", p=128)         # [128, 8, 1024]
    hdtT_ap = hdtT_d.ap().rearrange("(c p) b -> p c b", p=128)   # [128, 8, 4]

    with tile.TileContext(nc) as tc:
        with (
            tc.tile_pool(name="wpool", bufs=1) as wpool,
            tc.tile_pool(name="qpool", bufs=1) as qpool,
            tc.tile_pool(name="hpool", bufs=4) as hpool,
            tc.tile_pool(name="spool", bufs=2) as spool,
            tc.tile_pool(name="ppool", bufs=2) as ppool,
            tc.tile_pool(name="epool", bufs=2) as epool,
            tc.tile_pool(name="fpool", bufs=2) as fpool,
            tc.tile_pool(name="psq", bufs=1, space="PSUM") as psq,
            tc.tile_pool(name="psn", bufs=2, space="PSUM") as psn,
            tc.tile_pool(name="psd", bufs=1, space="PSUM") as psd,
        ):
            for _rep in range(reps):
                # ---- qW = h_d_t @ W for the local batches ----
                hdt_sb = wpool.tile([128, 8, B_LOC], f32, tag="hdt")
                nc.gpsimd.dma_start(hdt_sb[:], hdtT_ap)
                w_sb = wpool.tile([128, 8, D], f32, tag="w")
                nc.gpsimd.dma_start(w_sb[:], w_ap)
                qw_ps = psq.tile([B_LOC, D], f32, tag="qwps")
                for c in range(8):
                    for j in range(2):
                        nc.tensor.matmul(
                            qw_ps[:, j * 512:(j + 1) * 512],
                            hdt_sb[:, c, :],
                            w_sb[:, c, j * 512:(j + 1) * 512],
                            start=(c == 0),
                            stop=(c == 7),
                        )
                qw_sb = qpool.tile([B_LOC, D], f32, tag="qwsb")
                nc.scalar.copy(qw_sb[:], qw_ps[:])
                # replicate each batch's qW row across all 128 partitions
                # (partition_broadcast needs its source at partition 0, so
                # stage each row through a partition-0 tile via SBUF DMA)
                qwb = qpool.tile([128, B_LOC * D], f32, tag="qwb")
                for b in range(B_LOC):
                    qrow = ppool.tile([1, D], f32, tag="qrow", bufs=1)
                    nc.gpsimd.dma_start(qrow[:], qw_sb[b:b + 1, :])
                    nc.gpsimd.partition_broadcast(
                        qwb[:, b * D:(b + 1) * D], qrow[:]
                    )
                ones_sb = qpool.tile([128, 1], f32, tag="ones")
                nc.vector.memset(ones_sb[:], 1.0)

                # ---- main single pass over the cache ----
                g = 0  # global slab counter for ring alternation
                for b in range(B_LOC):
                    e_b = epool.tile([128, NT], f32, tag="eb")
                    num_ps = psn.tile([1, D], f32, tag="num")
                    slabs = SLABS_LAST if b == B_LOC - 1 else SLABS_STD
                    i0 = 0
                    for ntile in slabs:
                        h_t = hpool.tile([TT, ntile, D], f32r, tag="h")
                        eng = DMA_ENGS[g % len(DMA_ENGS)](nc)
                        eng.dma_start(h_t[:], h_ap[b, :, i0:i0 + ntile, :])
                        g += 1
                        # score -> exp -> matmul in EXP_GRAN-tile groups so
                        # PE work interleaves with the DVE score stream
                        # (keeps the HAM clock warm, shortens slab lifetime)
                        for j0 in range(0, ntile, EXP_GRAN):
                            gsz = min(EXP_GRAN, ntile - j0)
                            for j in range(j0, j0 + gsz):
                                i = i0 + j
                                prod = spool.tile([TT, D], f32, tag="prodv")
                                nc.vector.scalar_tensor_tensor(
                                    out=prod[:],
                                    in0=h_t[:, j, :],
                                    scalar=1.0,
                                    in1=qwb[:, b * D:(b + 1) * D],
                                    op0=mybir.AluOpType.mult,
                                    op1=mybir.AluOpType.mult,
                                    accum_out=e_b[:, i:i + 1],
                                )
                            p_t = ppool.tile([TT, gsz], f32r, tag="pt")
                            nc.scalar.activation(
                                p_t[:],
                                e_b[:, i0 + j0:i0 + j0 + gsz],
                                mybir.ActivationFunctionType.Exp,
                            )
                            for j in range(j0, j0 + gsz):
                                i = i0 + j
                                for half in range(2):
                                    nc.tensor.matmul(
                                        num_ps[:, half * 512:(half + 1) * 512],
                                        p_t[:, j - j0:j - j0 + 1],
                                        h_t[:, j, half * 512:(half + 1) * 512],
                                        start=(i == 0),
                                        stop=(i == NT - 1),
                                    )
                        i0 += ntile
                    # ---- finalize batch b ----
                    e_red = fpool.tile([128, 1], f32, tag="ered")
                    nc.vector.tensor_reduce(
                        e_red[:], e_b[:], axis=mybir.AxisListType.X,
                        op=mybir.AluOpType.add,
                    )
                    den_ps = psd.tile([1, 1], f32, tag="den")
                    nc.tensor.matmul(
                        den_ps[:], e_red[:], ones_sb[:], start=True, stop=True
                    )
                    den_sb = fpool.tile([1, 1], f32, tag="densb")
                    nc.vector.tensor_scalar_add(den_sb[:], den_ps[:], EPS)
                    recip = fpool.tile([1, 1], f32, tag="recip")
                    nc.vector.reciprocal(recip[:], den_sb[:])
                    c_sb = fpool.tile([1, D], f32, tag="csb")
                    nc.vector.tensor_scalar_mul(c_sb[:], num_ps[:], recip[:])
                    nc.gpsimd.dma_start(c_d.ap()[b:b + 1, :], c_sb[:])

    nc.compile()
    return nc


def _get_module():
    if "nc" not in _NC_CACHE:
        _NC_CACHE["nc"] = _build_module()
    return _NC_CACHE["nc"]


def _make_in_maps(h_d_t, h_d_all, W):
    h_d_t = np.ascontiguousarray(np.asarray(h_d_t), dtype=np.float32)
    h_d_all = np.ascontiguousarray(np.asarray(h_d_all), dtype=np.float32)
    W = np.ascontiguousarray(np.asarray(W), dtype=np.float32)
    in_maps = []
    for i in range(N_CORES):
        sl = slice(i * B_LOC, (i + 1) * B_LOC)
        in_maps.append(
            {
                "h": h_d_all[sl],
                "hdtT": np.ascontiguousarray(h_d_t[sl].T),
                "W": W,
            }
        )
    return in_maps


def kernel(h_d_t, h_d_all, W, **run_kwargs):
    nc = _get_module()
    in_maps = _make_in_maps(h_d_t, h_d_all, W)
    res = bass_utils.run_bass_kernel_spmd(
        nc, in_maps, core_ids=list(range(N_CORES)), **run_kwargs
    )
    out = np.concatenate([res.results[i]["c"] for i in range(N_CORES)], axis=0)
    if run_kwargs:
        kernel.last_results = res
    return out


# revision 19
# speedup vs baseline: 1.0338x; 1.0338x over previous
"""Trainium2 Bass kernel for nn_DecoderAttention (bilinear-score attention).

Computes, for full inputs h_d_t [32,1024], h_d_all [32,4096,1024], W [1024,1024]:
    qW    = h_d_t @ W
    e     = einsum('bd,btd->bt', qW, h_d_all)
    alpha = exp(e) / (sum(e, axis=1) + 1e-8)
    c_t   = einsum('bt,btd->bd', alpha, h_d_all)

Strategy: data-parallel over batch — 4 batches per NeuronCore across 8 cores,
W replicated. Single pass over the cache, streamed as 4 MiB slabs (8 t-tiles
per DMA, tapering at the end of the stream) alternating between the two HWDGE
rings (ACT + SP) — measured fastest of {dual 221us, mono-SP 233us,
SP+SWDGE 249us} on HW; the old 512 KiB-per-DMA version ran at ~139 GB/s/core
vs ~360 here. h lives in DRAM and SBUF as float32r (bit-identical to f32) so
the TensorEngine fast path needs no per-tile conversion copy; DVE reads the
same bytes for the scores. Per 128-row t-tile, a fused DVE multiply+reduce
produces the raw scores, ACT exponentiates a slab's worth at a time, and the
TensorEngine accumulates exp(e)^T @ h into PSUM. The denominator is the
raw-score running sum, applied once per batch, so h_d_all is read exactly
once (64 MiB + 4 MiB of W per core ≈ 181 us of HBM at the measured
376 GB/s/core — the memory-roofline floor for this kernel).
"""

import numpy as np

import concourse.bass as bass  # noqa: F401  (engine types pulled via bacc)
import concourse.mybir as mybir
import concourse.tile as tile
from concourse import bacc, bass_utils

B, T, D = 32, 4096, 1024
N_CORES = 8
B_LOC = B // N_CORES  # 4 batches per core
TT = 128              # t-tile rows (matmul contraction dim)
NT = T // TT          # 32 t-tiles per batch
SLAB = 8              # t-tiles per DMA slab (4 MiB)
NSLAB = NT // SLAB    # 4 slabs per batch
GP_SPLIT = 0          # t-tiles per slab scored on GPSIMD (0: walrus rejects Pool STT)
# slab DMA queue: SP ring carries ONLY the h slab stream (in-order HWDGE
# rings head-of-line block on any DMA that waits for compute, so W/hdt/qrow/c
# all go via SWDGE instead)
DMA_ENGS = [lambda nc: nc.sync]
SLABS_STD = [8, 8, 8, 8]       # t-tiles per slab DMA, ordinary batches
SLABS_LAST = [8, 8, 8, 4, 2, 2]  # taper the final batch so the post-stream
                                 # tail is ~2 tiles of compute, not 8
EPS = 1e-8

_NC_CACHE = {}


def _build_module(reps=1):
    f32 = mybir.dt.float32
    f32r = mybir.dt.float32r

    nc = bacc.Bacc("TRN2", debug=False, num_devices=N_CORES)
    h_d = nc.dram_tensor("h", [B_LOC, T, D], f32r, kind="ExternalInput")
    hdtT_d = nc.dram_tensor("hdtT", [D, B_LOC], f32, kind="ExternalInput")
    w_d = nc.dram_tensor("W", [D, D], f32, kind="ExternalInput")
    c_d = nc.dram_tensor("c", [B_LOC, D], f32, kind="ExternalOutput")

    # tile-major view: [b, p, i, d] with t = i*TT + p; a slab DMA takes an
    # i-range so slab sizes can taper at the end of the stream
    h_ap = h_d.ap().rearrange("b (i p) d -> b p i d", p=TT)      # [4, 128, 32, 1024]
    w_ap = w_d.ap().rearrange("(c p) j -> p c j", p=128)         # [128, 8, 1024]
    hdtT_ap = hdtT_d.ap().rearrange("(c p) b -> p c b", p=128)   # [128, 8, 4]

    with tile.TileContext(nc) as tc:
        with (
            tc.tile_pool(name="wpool", bufs=1) as wpool,
            tc.tile_pool(name="qpool", bufs=1) as qpool,
            tc.tile_pool(name="hpool", bufs=4) as hpool,
            tc.tile_pool(name="spool", bufs=2) as spool,
            tc.tile_pool(name="ppool", bufs=2) as ppool,
            tc.tile_pool(name="epool", bufs=2) as epool,
            tc.tile_pool(name="fpool", bufs=2) as fpool,
            tc.tile_pool(name="psq", bufs=1, space="PSUM") as psq,
            tc.tile_pool(name="psn", bufs=2, space="PSUM") as psn,
            tc.tile_pool(name="psd", bufs=1, space="PSUM") as psd,
        ):
            for _rep in range(reps):
                # ---- qW = h_d_t @ W for the local batches ----
                hdt_sb = wpool.tile([128, 8, B_LOC], f32, tag="hdt")
                nc.gpsimd.dma_start(hdt_sb[:], hdtT_ap)
                w_sb = wpool.tile([128, 8, D], f32, tag="w")
                nc.gpsimd.dma_start(w_sb[:], w_ap)
                qw_ps = psq.tile([B_LOC, D], f32, tag="qwps")
                for c in range(8):
                    for j in range(2):
                        nc.tensor.matmul(
                            qw_ps[:, j * 512:(j + 1) * 512],
                            hdt_sb[:, c, :],
                            w_sb[:, c, j * 512:(j + 1) * 512],
                            start=(c == 0),
                            stop=(c == 7),
                        )
                qw_sb = qpool.tile([B_LOC, D], f32, tag="qwsb")
                nc.scalar.copy(qw_sb[:], qw_ps[:])
                # replicate each batch's qW row across all 128 partitions
                # (partition_broadcast needs its source at partition 0, so
                # stage each row through a partition-0 tile via SBUF DMA)
                qwb = qpool.tile([128, B_LOC * D], f32, tag="qwb")
                for b in range(B_LOC):
                    qrow = ppool.tile([1, D], f32, tag="qrow", bufs=1)
                    nc.gpsimd.dma_start(qrow[:], qw_sb[b:b + 1, :])
                    nc.gpsimd.partition_broadcast(
                        qwb[:, b * D:(b + 1) * D], qrow[:]
                    )
                ones_sb = qpool.tile([128, 1], f32, tag="ones")
                nc.vector.memset(ones_sb[:], 1.0)

                # ---- main single pass over the cache ----
                g = 0  # global slab counter for ring alternation
                for b in range(B_LOC):
                    e_b = epool.tile([128, NT], f32, tag="eb")
                    num_ps = psn.tile([1, D], f32, tag="num")
                    slabs = SLABS_LAST if b == B_LOC - 1 else SLABS_STD
                    i0 = 0
                    for ntile in slabs:
                        h_t = hpool.tile([TT, ntile, D], f32r, tag="h")
                        eng = DMA_ENGS[g % len(DMA_ENGS)](nc)
                        eng.dma_start(h_t[:], h_ap[b, :, i0:i0 + ntile, :])
                        g += 1
                        for j in range(ntile):
                            i = i0 + j
                            # fused multiply+reduce: prod = h*qW,
                            # e_b[:,i] = sum(prod); split 5:3 across the
                            # DVE and the otherwise-idle GPSIMD cluster
                            if j < GP_SPLIT:
                                seng = nc.gpsimd
                                prod = spool.tile([TT, D], f32, tag="prodg")
                            else:
                                seng = nc.vector
                                prod = spool.tile([TT, D], f32, tag="prodv")
                            seng.scalar_tensor_tensor(
                                out=prod[:],
                                in0=h_t[:, j, :],
                                scalar=1.0,
                                in1=qwb[:, b * D:(b + 1) * D],
                                op0=mybir.AluOpType.mult,
                                op1=mybir.AluOpType.mult,
                                accum_out=e_b[:, i:i + 1],
                            )
                        p_t = ppool.tile([TT, ntile], f32r, tag="pt")
                        nc.scalar.activation(
                            p_t[:],
                            e_b[:, i0:i0 + ntile],
                            mybir.ActivationFunctionType.Exp,
                        )
                        for j in range(ntile):
                            i = i0 + j
                            for half in range(2):
                                nc.tensor.matmul(
                                    num_ps[:, half * 512:(half + 1) * 512],
                                    p_t[:, j:j + 1],
                                    h_t[:, j, half * 512:(half + 1) * 512],
                                    start=(i == 0),
                                    stop=(i == NT - 1),
                                )
                        i0 += ntile
                    # ---- finalize batch b ----
                    e_red = fpool.tile([128, 1], f32, tag="ered")
                    nc.vector.tensor_reduce(
                        e_red[:], e_b[:], axis=mybir.AxisListType.X,
                        op=mybir.AluOpType.add,
                    )
                    den_ps = psd.tile([1, 1], f32, tag="den")
                    nc.tensor.matmul(
                        den_ps[:], e_red[:], ones_sb[:], start=True, stop=True
                    )
                    den_sb = fpool.tile([1, 1], f32, tag="densb")
                    nc.vector.tensor_scalar_add(den_sb[:], den_ps[:], EPS)
                    recip = fpool.tile([1, 1], f32, tag="recip")
                    nc.vector.reciprocal(recip[:], den_sb[:])
                    c_sb = fpool.tile([1, D], f32, tag="csb")
                    nc.vector.tensor_scalar_mul(c_sb[:], num_ps[:], recip[:])
                    nc.gpsimd.dma_start(c_d.ap()[b:b + 1, :], c_sb[:])

    nc.compile()
    return nc


def _get_module():
    if "nc" not in _NC_CACHE:
        _NC_CACHE["nc"] = _build_module()
    return _NC_CACHE["nc"]


def _make_in_maps(h_d_t, h_d_all, W):
    h_d_t = np.ascontiguousarray(np.asarray(h_d_t), dtype=np.float32)
    h_d_all = np.ascontiguousarray(np.asarray(h_d_all), dtype=np.float32)
    W = np.ascontiguousarray(np.asarray(W), dtype=np.float32)
    in_maps = []
    for i in range(N_CORES):
        sl = slice(i * B_LOC, (i + 1) * B_LOC)
        in_maps.append(
            {
                "h": h_d_all[sl],
                "hdtT": np.ascontiguousarray(h_d_t[sl].T),
                "W": W,
            }
        )
    return in_maps


def kernel(h_d_t, h_d_all, W, **run_kwargs):
    nc = _get_module()
    in_maps = _make_in_maps(h_d_t, h_d_all, W)
    res = bass_utils.run_bass_kernel_spmd(
        nc, in_maps, core_ids=list(range(N_CORES)), **run_kwargs
    )
    out = np.concatenate([res.results[i]["c"] for i in range(N_CORES)], axis=0)
    if run_kwargs:
        kernel.last_results = res
    return out
